# Initial kernel scaffold
#
"""Causal GQA self-attention (B=4, T=2048, C=2048, H=16, HKV=4, D=128) on 8 trn2 cores.

Sharding: core = (batch b = core//2) x (kv-head pair s = core%2).
Each core computes, for its batch and its 2 kv heads (8 q heads):
  k^T (f32r resident), v (fp16, direct [t, d] layout), q^T per head (f32r,
  SBUF-resident, 2-head rotation), causal attention in transposed layout
  (S^T blocks [tk=128, tq=512], f32r matmuls), exp on Act with fp16 probs,
  post-exp multiplicative 0/1 masks on GpSimd, AV in fp16, row sums via
  fp16 DVE slot accumulation + 2 small ones-matmuls, per-(h,qt) late
  normalization (reciprocal + partition_broadcast), partial c_proj
  (row-slice of Wc, f32r) -> [T, C] fp16 partial output.
Host sums the two partials per batch and adds bc.
Attention for head h is interleaved with the q-projection of head h+1 so the
Act-engine exp work hides under projection matmuls.
"""

import math
from contextlib import ExitStack

import numpy as np

B, T, C = 4, 2048, 2048
HKV, D, G = 4, 128, 4
NCORES = 8
HPC = 8            # q heads per core
KVPC = 2           # kv heads per core
TQ = 512           # q-tile (free dim of S^T blocks)
NTQ = T // TQ      # 4
NKB = T // 128     # 16 k-blocks
SCALE = 1.0 / math.sqrt(D)

_NC = None


def _round_f32r(a):
    """Round fp32 to f32r (8-bit exp, 11-bit mantissa) to match PE input rounding."""
    u = np.ascontiguousarray(a, dtype=np.float32).view(np.uint32)
    add = ((u >> np.uint32(12)) & np.uint32(1)) + np.uint32(0x7FF)
    u = (u + add) & np.uint32(0xFFFFF000)
    return u.view(np.float32)


def _make_masks():
    """Multiplicative 0/1 masks, applied to probs after exp.
    maskA = [p0|p1], maskB = [p2|p3]: p-image[tk, tq] = 1 iff tq >= tk + 128*p."""
    i = np.arange(128)[:, None]
    c = np.arange(512)[None, :]
    imgs = [(c >= i + 128 * p).astype(np.float32) for p in range(4)]
    maskA = np.concatenate([imgs[0], imgs[1]], axis=1)
    maskB = np.concatenate([imgs[2], imgs[3]], axis=1)
    return maskA.astype(np.float16), maskB.astype(np.float16)


def _emit(tc, io):
    from concourse import mybir

    nc = tc.nc
    F32 = mybir.dt.float32
    F32R = mybir.dt.float32r
    FP16 = mybir.dt.float16
    EXP = mybir.ActivationFunctionType.Exp
    ADD = mybir.AluOpType.add
    MULT = mybir.AluOpType.mult

    ctx = ExitStack()
    with ctx:
        persist = ctx.enter_context(tc.tile_pool(name="persist", bufs=1))
        kT_sb = persist.tile([128, KVPC * T], F32R, name="kT", tag="kT")   # [d, kv*T + t]
        v_sb = persist.tile([128, NKB * 256], FP16, name="v", tag="v")     # [t%128, tb*256 + kv*128 + d]
        ones_sb = persist.tile([128, 8], FP16, name="ones", tag="ones")
        maskA = persist.tile([128, 1024], FP16, name="maskA", tag="maskA")
        maskB = persist.tile([128, 1024], FP16, name="maskB", tag="maskB")
        nc.sync.dma_start(ones_sb[:], io["ones"])
        nc.gpsimd.dma_start(maskA[:], io["maskA"])
        nc.gpsimd.dma_start(maskB[:], io["maskB"])

        # long-lived attention-side pools
        yres = ctx.enter_context(tc.tile_pool(name="yres", bufs=8))
        yT_all = [yres.tile([128, T], F32R, name="yT", tag="yT") for _ in range(HPC)]
        qpool = ctx.enter_context(tc.tile_pool(name="qpool", bufs=2))
        pTpool = ctx.enter_context(tc.tile_pool(name="pTp", bufs=3))
        rsaccp = ctx.enter_context(tc.tile_pool(name="rsaccp", bufs=2))
        rsstp = ctx.enter_context(tc.tile_pool(name="rsstp", bufs=1))
        rsinvp = ctx.enter_context(tc.tile_pool(name="rsinvp", bufs=1))
        binvp = ctx.enter_context(tc.tile_pool(name="binvp", bufs=1))

        # PSUM pools (8 banks total): psS 3x[128,1024] (6) + psY 1 + psRS 1
        psctx = ExitStack()
        psS = psctx.enter_context(tc.tile_pool(name="psS", bufs=3, space="PSUM"))
        psY = psctx.enter_context(tc.tile_pool(name="psY", bufs=1, space="PSUM"))
        psRS = psctx.enter_context(tc.tile_pool(name="psRS", bufs=1, space="PSUM"))

        qT_heads = {}
        wq_blocks = {}

        # ---------------- attention emission helpers ----------------
        pending = []   # deferred tail work, flushed inside the next block

        def flush_pending():
            while pending:
                pending.pop(0)()

        def emit_rs_finish(rs_acc, h, qt):
            # ones-matmul row sums + reciprocal + broadcast + normalize yT slice
            def work():
                rs_ps = psRS.tile([1, TQ], F32, name="rsps", tag="rsps")
                for j in range(2):
                    nc.tensor.matmul(rs_ps[:],
                                     lhsT=ones_sb[:, 0:1],
                                     rhs=rs_acc[:, j * TQ:(j + 1) * TQ],
                                     start=(j == 0), stop=(j == 1))
                rs_st = rsstp.tile([1, TQ], F32, name="rsst", tag="rsst")
                nc.scalar.copy(rs_st[:], rs_ps[:])
                rsinv = rsinvp.tile([1, TQ], FP16, name="rsinv", tag="rsinv")
                with nc.allow_low_precision("fp16 1/rowsum is plenty for 2e-2"):
                    nc.vector.reciprocal(rsinv[:], rs_st[:])
                binv = binvp.tile([128, TQ], FP16, name="binv", tag="binv")
                nc.gpsimd.partition_broadcast(binv[:], rsinv[:], channels=128)
                nc.vector.tensor_tensor(
                    out=yT_all[h][:, qt * TQ:(qt + 1) * TQ],
                    in0=yT_all[h][:, qt * TQ:(qt + 1) * TQ],
                    in1=binv[:], op=MULT)
            pending.append(work)

        def emit_att_qt(h, qt):
            kv = h // G
            qT_h = qT_heads[h]
            nkb = (qt + 1) * (TQ // 128)
            ng = nkb // 2
            y_ps = psY.tile([128, TQ], F32, name="yps", tag="yps")
            rs_acc = rsaccp.tile([128, 1024], FP16, name="rsacc", tag="rsacc")

            sgs = {}
            pTs = {}

            def emit_S(g):
                sg = psS.tile([128, 1024], F32, name="sg", tag="sg")
                for j in range(2):
                    kb = g * 2 + j
                    nc.tensor.matmul(
                        sg[:, j * TQ:(j + 1) * TQ],
                        lhsT=kT_sb[:, kv * T + kb * 128:kv * T + (kb + 1) * 128],
                        rhs=qT_h[:, qt * TQ:(qt + 1) * TQ],
                        start=True, stop=True)
                sgs[g] = sg

            def emit_E(g):
                pT = pTpool.tile([128, 1024], FP16, name="pT", tag="pT")
                nc.scalar.activation(pT[:], sgs.pop(g)[:], EXP, scale=SCALE)
                if g >= 2 * qt:           # diagonal group: zero the invalid probs
                    m = maskA if g == 2 * qt else maskB
                    nc.gpsimd.tensor_tensor(out=pT[:], in0=pT[:], in1=m[:], op=MULT)
                pTs[g] = pT

            def emit_AV(g):
                pT = pTs.pop(g)
                for j in range(2):
                    kb = g * 2 + j
                    nc.tensor.matmul(
                        y_ps[:],
                        lhsT=v_sb[:, kb * 256 + kv * 128:kb * 256 + (kv + 1) * 128],
                        rhs=pT[:, j * TQ:(j + 1) * TQ],
                        start=(kb == 0), stop=(kb == nkb - 1))
                if g == 0:
                    nc.vector.tensor_copy(rs_acc[:], pT[:])
                else:
                    nc.vector.tensor_tensor(out=rs_acc[:], in0=rs_acc[:],
                                            in1=pT[:], op=ADD)

            emit_S(0)
            emit_E(0)
            flush_pending()
            if ng > 1:
                emit_S(1)
                emit_E(1)
            for g in range(2, ng):
                emit_S(g)
                emit_AV(g - 2)
                emit_E(g)
            if ng > 1:
                emit_AV(ng - 2)
            emit_AV(ng - 1)
            nc.vector.tensor_copy(yT_all[h][:, qt * TQ:(qt + 1) * TQ], y_ps[:])
            emit_rs_finish(rs_acc, h, qt)

        # ---------------- Phase A + interleaved attention ----------------
        with (
            tc.tile_pool(name="xpool", bufs=16) as xpool,
            tc.tile_pool(name="wstream", bufs=3) as wstream,
        ):
            wq3d = io["wq"].rearrange("(cb r) c -> r cb c", r=128)   # [128, 16, 1024]

            def load_wq(hv):
                hd, half = divmod(hv, 2)
                t = wstream.tile([128, 8, 128], FP16, name="wqb", tag="wqb", bufs=3)
                nc.sync.dma_start(
                    t[:], wq3d[:, half * 8:(half + 1) * 8,
                               hd * 128:(hd + 1) * 128])
                return t

            x_sb = []
            _wq_sched = {0: 0, 7: 1, 11: 2}
            for cb in range(16):
                xt = xpool.tile([128, T], FP16, name="x", tag="x")
                nc.sync.dma_start(xt[:], io["xT"][cb * 128:(cb + 1) * 128, :])
                x_sb.append(xt)
                if cb in _wq_sched:
                    wq_blocks[_wq_sched[cb]] = load_wq(_wq_sched[cb])

            # k projection: k^T[d, t] resident (f32r)
            with tc.tile_pool(name="wkres", bufs=1) as wkres:
                wk_sb = wkres.tile([128, 16 * 256], FP16, name="wk", tag="wk")
                nc.sync.dma_start(
                    wk_sb[:].rearrange("r (cb c) -> r cb c", c=256),
                    io["wk"].rearrange("(cb r) c -> r cb c", r=128))
                for kvi in range(KVPC):
                    for t4 in range(NTQ):
                        ps_k = psS.tile([128, TQ], F32, name="kps", tag="sg")
                        for cb in range(16):
                            nc.tensor.matmul(
                                ps_k[:],
                                lhsT=wk_sb[:, cb * 256 + kvi * 128:cb * 256 + (kvi + 1) * 128],
                                rhs=x_sb[cb][:, t4 * TQ:(t4 + 1) * TQ],
                                start=(cb == 0), stop=(cb == 15))
                        nc.vector.tensor_copy(
                            kT_sb[:, kvi * T + t4 * TQ:kvi * T + (t4 + 1) * TQ],
                            ps_k[:])

            # v projection, direct [t, vd] layout (fp16)
            with tc.tile_pool(name="wvres", bufs=1) as wvres:
                wv_sb = wvres.tile([128, 16 * 256], FP16, name="wv", tag="wv")
                nc.sync.dma_start(
                    wv_sb[:].rearrange("r (cb c) -> r cb c", c=256),
                    io["wv"].rearrange("(cb r) c -> r cb c", r=128))
                for tb in range(16):
                    ps_v = psS.tile([128, 256], F32, name="vps", tag="sg")
                    for cb in range(16):
                        nc.tensor.matmul(
                            ps_v[:],
                            lhsT=x_sb[cb][:, tb * 128:(tb + 1) * 128],
                            rhs=wv_sb[:, cb * 256:(cb + 1) * 256],
                            start=(cb == 0), stop=(cb == 15))
                    nc.vector.tensor_copy(v_sb[:, tb * 256:(tb + 1) * 256], ps_v[:])

            def emit_q_chunk(h, t4):
                # one quarter of head h's q projection: q^T[hd, t4*TQ:...]
                if t4 == 0:
                    qT_heads[h] = qpool.tile([128, T], F32R, name="qT", tag="qT")
                    wq_blocks[(h, "halves")] = [wq_blocks.pop(h * 2),
                                                wq_blocks.pop(h * 2 + 1)]
                    for hv in (h * 2 + 3, h * 2 + 4):
                        if 2 <= hv < HPC * 2 and hv not in wq_blocks:
                            wq_blocks[hv] = load_wq(hv)
                wq_halves = wq_blocks[(h, "halves")]
                ps_q = psS.tile([128, TQ], F32, name="qps", tag="sg")
                for cb in range(16):
                    nc.tensor.matmul(ps_q[:],
                                     lhsT=wq_halves[cb // 8][:, cb % 8, :],
                                     rhs=x_sb[cb][:, t4 * TQ:(t4 + 1) * TQ],
                                     start=(cb == 0), stop=(cb == 15))
                nc.vector.tensor_copy(
                    qT_heads[h][:, t4 * TQ:(t4 + 1) * TQ], ps_q[:])
                if t4 == NTQ - 1:
                    wq_blocks.pop((h, "halves"))

            for t4 in range(NTQ):
                emit_q_chunk(0, t4)

            for h in range(HPC - 1):
                for qt in range(NTQ):
                    emit_q_chunk(h + 1, qt)
                    emit_att_qt(h, qt)
                qT_heads.pop(h)

        # xpool closed; wc loads into freed space, attention h=7 runs meanwhile
        wcres = ctx.enter_context(tc.tile_pool(name="wcres", bufs=1))
        wc_sb = [wcres.tile([128, C], F32R, name="wc", tag=f"wc{h}")
                 for h in range(HPC)]
        for h in range(HPC):
            nc.sync.dma_start(wc_sb[h][:],
                              io["wc"][h * 128:(h + 1) * 128, :].bitcast(F32R))
        for qt in range(NTQ):
            emit_att_qt(HPC - 1, qt)
        flush_pending()
        psctx.close()

        # ---------------- Phase C: output projection ----------------
        with (
            tc.tile_pool(name="ostage", bufs=2) as ostage,
            tc.tile_pool(name="psC", bufs=8, space="PSUM") as psC,
        ):
            for tq in range(16):
                ops = [psC.tile([128, 512], F32, name="ops", tag="ops") for _ in range(4)]
                for h in range(HPC):
                    for cp in range(4):
                        nc.tensor.matmul(
                            ops[cp][:],
                            lhsT=yT_all[h][:, tq * 128:(tq + 1) * 128],
                            rhs=wc_sb[h][:, cp * 512:(cp + 1) * 512],
                            start=(h == 0), stop=(h == HPC - 1))
                ost = ostage.tile([128, C], FP16, name="ost", tag="ost")
                for cp in range(4):
                    if cp < 2:
                        nc.scalar.copy(ost[:, cp * 512:(cp + 1) * 512], ops[cp][:])
                    else:
                        nc.vector.tensor_copy(ost[:, cp * 512:(cp + 1) * 512],
                                              ops[cp][:])
                nc.scalar.dma_start(io["out"][tq * 128:(tq + 1) * 128, :], ost[:])


def _build_nc():
    import concourse.tile as tile
    from concourse import bacc, mybir

    F32 = mybir.dt.float32
    FP16 = mybir.dt.float16
    nc = bacc.Bacc("TRN2", target_bir_lowering=False, debug=False,
                   num_devices=NCORES)
    io = {
        "xT": nc.dram_tensor("xT", [C, T], FP16, kind="ExternalInput").ap(),
        "wq": nc.dram_tensor("wq", [C, HPC * 128], FP16, kind="ExternalInput").ap(),
        "wk": nc.dram_tensor("wk", [C, KVPC * 128], FP16, kind="ExternalInput").ap(),
        "wv": nc.dram_tensor("wv", [C, KVPC * 128], FP16, kind="ExternalInput").ap(),
        "wc": nc.dram_tensor("wc", [HPC * 128, C], F32, kind="ExternalInput").ap(),
        "maskA": nc.dram_tensor("maskA", [128, 1024], FP16, kind="ExternalInput").ap(),
        "maskB": nc.dram_tensor("maskB", [128, 1024], FP16, kind="ExternalInput").ap(),
        "ones": nc.dram_tensor("ones", [128, 8], FP16, kind="ExternalInput").ap(),
        "out": nc.dram_tensor("out", [T, C], FP16, kind="ExternalOutput").ap(),
    }
    with tile.TileContext(nc) as tc:
        _emit(tc, io)
    nc.compile()
    return nc


def _get_nc():
    global _NC
    if _NC is None:
        _NC = _build_nc()
    return _NC


def make_in_maps(x, Wq, Wkv, Wc):
    BF = np.float16
    x = np.asarray(x, dtype=np.float32)
    Wq = np.asarray(Wq, dtype=np.float32)
    Wkv = np.asarray(Wkv, dtype=np.float32)
    Wc = np.asarray(Wc, dtype=np.float32)
    maskA, maskB = _make_masks()
    ones = np.ones((128, 8), dtype=BF)
    in_maps = []
    for core in range(NCORES):
        b, s = core // 2, core % 2
        in_maps.append({
            "xT": np.ascontiguousarray(x[b].T).astype(BF),
            "wq": Wq[:, s * 1024:(s + 1) * 1024].astype(BF),
            "wk": Wkv[:, s * 256:(s + 1) * 256].astype(BF),
            "wv": Wkv[:, 512 + s * 256:512 + (s + 1) * 256].astype(BF),
            "wc": _round_f32r(Wc[s * 1024:(s + 1) * 1024, :]),
            "maskA": maskA,
            "maskB": maskB,
            "ones": ones,
        })
    return in_maps


def combine_outputs(results, bc):
    bc = np.asarray(bc, dtype=np.float32)
    out = np.empty((B, T, C), dtype=np.float32)
    for b in range(B):
        out[b] = (results[2 * b]["out"].astype(np.float32)
                  + results[2 * b + 1]["out"].astype(np.float32))
    out += bc[None, None, :]
    return out


def kernel(x, Wq, Wkv, Wc, bc):
    from concourse.bass_utils import run_bass_kernel_spmd

    nc = _get_nc()
    in_maps = make_in_maps(x, Wq, Wkv, Wc)
    res = run_bass_kernel_spmd(nc, in_maps, list(range(NCORES)))
    return combine_outputs(res.results, bc)



# revision 67
# speedup vs baseline: 2.3297x; 2.3297x over previous
"""Causal GQA self-attention (B=4, T=2048, C=2048, H=16, HKV=4, D=128) on 8 trn2 cores.

Sharding: core = (batch b = core//2) x (kv-head pair s = core%2).
Each core computes, for its batch and its 2 kv heads (8 q heads), all-fp16
matmul inputs throughout:
  k/v projections in cb-major waves over 8 bank-exclusive PSUM chains so the
  projection work is absorbed into the x DMA window; q^T per head
  (SBUF-resident, 2-head rotation); causal attention in transposed layout
  (S^T blocks [tk=128, tq=512]) with the diagonal band narrowed to its valid
  widths (512/384/256/128); exp on Act with fp16 probs; post-exp
  multiplicative triangle masks on DVE; AV in fp16; row sums via fp16 DVE
  accumulation + ones[128,128]-matmuls (fused reduce+broadcast) and
  reciprocal_approx_fast; head-7 attention fused with the c_proj so its exp
  work hides under c_proj matmuls -> [T, C] fp16 partial output.
Host sums the two partials per batch and adds bc. All weights/x are
pre-transposed host-side so every DMA is contiguous (no descriptor storms).
Attention for head h is interleaved with the q-projection of head h+1 so the
Act-engine exp work hides under projection matmuls. GpSimd is kept idle
(its ucode library reloads stall all engines ~7us each).
"""

import math
from contextlib import ExitStack

import numpy as np

B, T, C = 4, 2048, 2048
HKV, D, G = 4, 128, 4
NCORES = 8
HPC = 8            # q heads per core
KVPC = 2           # kv heads per core
TQ = 512           # q-tile (free dim of S^T blocks)
NTQ = T // TQ      # 4
NKB = T // 128     # 16 k-blocks
SCALE = 1.0 / math.sqrt(D)

_NC = None


def _make_masks():
    """Multiplicative 0/1 masks for the narrowed diagonal band, applied to
    probs after exp. Each kb sub-block is computed at its valid width with a
    tq offset that aligns its leading 128x128 triangle: 1 iff col >= row."""
    i = np.arange(128)[:, None]

    def band(w):
        c = np.arange(w)[None, :]
        return (c >= i).astype(np.float16)

    maskA = np.concatenate([band(512), band(384)], axis=1)   # p0 | p1
    maskB = np.concatenate([band(256), band(128)], axis=1)   # p2 | p3
    return maskA, maskB


def _emit(tc, io):
    from concourse import mybir

    nc = tc.nc
    F32 = mybir.dt.float32
    FP16 = mybir.dt.float16
    EXP = mybir.ActivationFunctionType.Exp
    ADD = mybir.AluOpType.add
    MULT = mybir.AluOpType.mult

    ctx = ExitStack()
    with ctx:
        persist = ctx.enter_context(tc.tile_pool(name="persist", bufs=1))
        kT_sb = persist.tile([128, KVPC * T], FP16, name="kT", tag="kT")   # [d, kv*T + t]
        v_sb = persist.tile([128, NKB * 256], FP16, name="v", tag="v")     # [t%128, tb*256 + kv*128 + d]
        ones_sb = persist.tile([128, 128], FP16, name="ones", tag="ones")
        maskA = persist.tile([128, 896], FP16, name="maskA", tag="maskA")
        maskB = persist.tile([128, 384], FP16, name="maskB", tag="maskB")
        nc.scalar.dma_start(maskA[:], io["maskA"])
        nc.scalar.dma_start(maskB[:], io["maskB"])
        nc.scalar.dma_start(ones_sb[:], io["ones"])

        # long-lived attention-side pools
        yres = ctx.enter_context(tc.tile_pool(name="yres", bufs=8))
        yT_all = [yres.tile([128, T], FP16, name="yT", tag="yT") for _ in range(HPC)]
        qpool = ctx.enter_context(tc.tile_pool(name="qpool", bufs=2))
        pTpool = ctx.enter_context(tc.tile_pool(name="pTp", bufs=4))
        rsaccp = ctx.enter_context(tc.tile_pool(name="rsaccp", bufs=2))
        binvp = ctx.enter_context(tc.tile_pool(name="binvp", bufs=2))

        # PSUM pools (8 banks total): psY 1 + psRS 1 + psS 3x[128,1024] (6).
        # psS opens last so it can be swapped for psS2(4) + psC(2) at head 7.
        psctx = ExitStack()
        psY = psctx.enter_context(tc.tile_pool(name="psY", bufs=1, space="PSUM"))
        psRS = psctx.enter_context(tc.tile_pool(name="psRS", bufs=1, space="PSUM"))
        psSctx = ExitStack()
        psS = psSctx.enter_context(tc.tile_pool(name="psS", bufs=3, space="PSUM"))
        sgpool = [psS]

        qT_heads = {}
        wq_blocks = {}

        # ---------------- attention emission helpers ----------------
        pending = []   # deferred tail work, flushed inside the next block

        def flush_pending():
            while pending:
                pending.pop(0)()

        def emit_rs_finish(rs_acc, h, qt):
            # ones[128,128]-matmul: every PSUM partition gets the full rowsum
            # (fused reduce + partition-broadcast), then reciprocal + normalize.
            def work():
                rs_ps = psRS.tile([128, TQ], F32, name="rsps", tag="rsps")
                if qt == 0:
                    # half1 cols 0:128 were never written (narrowed diagonal)
                    nc.tensor.matmul(rs_ps[:], lhsT=ones_sb[:],
                                     rhs=rs_acc[:, 0:TQ],
                                     start=True, stop=False)
                    nc.tensor.matmul(rs_ps[:, 128:TQ], lhsT=ones_sb[:],
                                     rhs=rs_acc[:, TQ + 128:2 * TQ],
                                     start=False, stop=True)
                else:
                    for j in range(2):
                        nc.tensor.matmul(rs_ps[:],
                                         lhsT=ones_sb[:],
                                         rhs=rs_acc[:, j * TQ:(j + 1) * TQ],
                                         start=(j == 0), stop=(j == 1))
                binv = binvp.tile([128, TQ], F32, name="binv", tag="binv")
                nc.vector.reciprocal_approx_fast(binv[:], rs_ps[:])
                nc.vector.tensor_tensor(
                    out=yT_all[h][:, qt * TQ:(qt + 1) * TQ],
                    in0=yT_all[h][:, qt * TQ:(qt + 1) * TQ],
                    in1=binv[:], op=MULT)
            pending.append(work)

        def emit_att_qt(h, qt):
            kv = h // G
            qT_h = qT_heads[h]
            nkb = (qt + 1) * (TQ // 128)
            ng = nkb // 2
            y_ps = psY.tile([128, TQ], F32, name="yps", tag="yps")
            rs_acc = rsaccp.tile([128, 1024], FP16, name="rsacc", tag="rsacc")

            sgs = {}
            pTs = {}

            def parts_of(g):
                # (kb, col0, width, tq_off, rs_col0) per kb sub-block; the
                # last two groups are the causal diagonal band, narrowed to
                # their valid widths (fp16 matmul is full-rate at any width).
                if g == ng - 2:
                    return [(4 * qt + 0, 0, 512, 0, 0),
                            (4 * qt + 1, 512, 384, 128, TQ + 128)], maskA
                if g == ng - 1:
                    return [(4 * qt + 2, 0, 256, 256, 256),
                            (4 * qt + 3, 256, 128, 384, TQ + 384)], maskB
                return [(2 * g, 0, 512, 0, 0),
                        (2 * g + 1, 512, 512, 0, TQ)], None

            def emit_S(g):
                parts, _ = parts_of(g)
                sg = sgpool[0].tile([128, 1024], F32, name="sg", tag="sg")
                for kb, col0, w, tq_off, _rs in parts:
                    nc.tensor.matmul(
                        sg[:, col0:col0 + w],
                        lhsT=kT_sb[:, kv * T + kb * 128:kv * T + (kb + 1) * 128],
                        rhs=qT_h[:, qt * TQ + tq_off:(qt + 1) * TQ],
                        start=True, stop=True)
                sgs[g] = sg

            def emit_E(g):
                parts, m = parts_of(g)
                W = parts[-1][1] + parts[-1][2]
                pT = pTpool.tile([128, 1024], FP16, name="pT", tag="pT")
                nc.scalar.activation(pT[:, 0:W], sgs.pop(g)[:, 0:W], EXP,
                                     scale=SCALE)
                if m is not None:        # diagonal band: zero invalid probs
                    nc.vector.tensor_tensor(out=pT[:, 0:W], in0=pT[:, 0:W],
                                            in1=m[:], op=MULT)
                pTs[g] = pT

            def emit_AV(g):
                parts, _ = parts_of(g)
                pT = pTs.pop(g)
                for kb, col0, w, tq_off, _rs in parts:
                    nc.tensor.matmul(
                        y_ps[:, tq_off:TQ],
                        lhsT=v_sb[:, kb * 256 + kv * 128:kb * 256 + (kv + 1) * 128],
                        rhs=pT[:, col0:col0 + w],
                        start=(kb == 0), stop=(kb == nkb - 1))
                for kb, col0, w, tq_off, rs0 in parts:
                    if g == 0 and kb < 2:
                        nc.vector.tensor_copy(rs_acc[:, rs0:rs0 + w],
                                              pT[:, col0:col0 + w])
                    else:
                        nc.vector.tensor_tensor(
                            out=rs_acc[:, rs0:rs0 + w],
                            in0=rs_acc[:, rs0:rs0 + w],
                            in1=pT[:, col0:col0 + w], op=ADD)

            emit_S(0)
            emit_E(0)
            flush_pending()
            if ng > 1:
                emit_S(1)
                emit_E(1)
            for g in range(2, ng):
                emit_S(g)
                emit_AV(g - 2)
                emit_E(g)
            if ng > 1:
                emit_AV(ng - 2)
            emit_AV(ng - 1)
            nc.vector.tensor_copy(yT_all[h][:, qt * TQ:(qt + 1) * TQ], y_ps[:])
            emit_rs_finish(rs_acc, h, qt)

        # ---------------- Phase A + interleaved attention ----------------
        wcres = ctx.enter_context(tc.tile_pool(name="wcres", bufs=1))
        wc_sb = [wcres.tile([128, C], FP16, name="wc", tag=f"wc{h}")
                 for h in range(HPC)]

        with (
            tc.tile_pool(name="xpool", bufs=1) as xpool,
            tc.tile_pool(name="wstream", bufs=3) as wstream,
        ):
            kvctx = ExitStack()
            wkres = kvctx.enter_context(tc.tile_pool(name="wkres", bufs=1))
            wvres = kvctx.enter_context(tc.tile_pool(name="wvres", bufs=1))

            def load_wq(hv):
                t = wstream.tile([128, 8, 128], FP16, name="wqb", tag="wqb", bufs=3)
                nc.sync.dma_start(
                    t[:].rearrange("r a c -> r (a c)"),
                    io["wq"][hv * 128:(hv + 1) * 128, :])
                return t

            # (host pre-transposes wk/wv/wq so these DMAs are contiguous)
            wk_sb = wkres.tile([128, 16 * 256], FP16, name="wk", tag="wk")



            # x: 32 half-tile DMAs in consumption order on one queue; finer
            # completion quanta let phase-A matmuls start on partial tiles.
            wv_sb = wvres.tile([128, 16 * 256], FP16, name="wv", tag="wv")
            nc.gpsimd.dma_start(wv_sb[:], io["wv"])
            HT = T // 2
            x_half = []
            for cb in range(16):
                x_half.append([xpool.tile([128, HT], FP16, name="x",
                                          tag=f"x{cb}h{j}") for j in range(2)])

            def xdma(cb, j):
                nc.sync.dma_start(x_half[cb][j][:],
                                  io["xT"][:, cb * T + j * HT:cb * T + (j + 1) * HT])

            xdma(0, 0)
            nc.sync.dma_start(wk_sb[:], io["wk"])
            xdma(0, 1)
            for cb in range(1, 16):
                xdma(cb, 0)
                xdma(cb, 1)

            def xs(cb, lo, hi):
                j = lo // HT
                assert hi <= (j + 1) * HT
                return x_half[cb][j][:, lo - j * HT:hi - j * HT]

            for i in range(3):
                wq_blocks[i] = load_wq(i)

            # k+v projections in waves, cb-major: 8 concurrent bank-exclusive
            # PSUM chains consume each x tile as it lands, absorbing most of
            # the projection work into the x DMA window.
            def emit_kv_wave(kvi, tbs):
                kp = ([psS.tile([128, 1024], F32, name="kp", tag="sg")
                       for _ in range(2)] if kvi is not None else [])
                vp = psS.tile([128, 1024], F32, name="vp", tag="sg")
                vy = psY.tile([128, TQ], F32, name="vy", tag="yps")
                vr = psRS.tile([128, TQ], F32, name="vr", tag="rsps")
                vslots = [(vp, 0), (vp, 512), (vy, 0), (vr, 0)]
                for cb in range(16):
                    for i, tb in enumerate(tbs):
                        tile, col = vslots[i]
                        nc.tensor.matmul(
                            tile[:, col:col + 256],
                            lhsT=xs(cb, tb * 128, (tb + 1) * 128),
                            rhs=wv_sb[:, cb * 256:(cb + 1) * 256],
                            start=(cb == 0), stop=(cb == 15))
                    if kvi is not None:
                        for t4 in range(NTQ):
                            tile, col = kp[t4 // 2], (t4 % 2) * 512
                            nc.tensor.matmul(
                                tile[:, col:col + 512],
                                lhsT=wk_sb[:, cb * 256 + kvi * 128:cb * 256 + (kvi + 1) * 128],
                                rhs=xs(cb, t4 * TQ, (t4 + 1) * TQ),
                                start=(cb == 0), stop=(cb == 15))
                if kvi is not None:
                    for t4 in range(NTQ):
                        tile, col = kp[t4 // 2], (t4 % 2) * 512
                        nc.vector.tensor_copy(
                            kT_sb[:, kvi * T + t4 * TQ:kvi * T + (t4 + 1) * TQ],
                            tile[:, col:col + 512])
                for i, tb in enumerate(tbs):
                    tile, col = vslots[i]
                    if i % 2:
                        nc.scalar.copy(v_sb[:, tb * 256:(tb + 1) * 256],
                                       tile[:, col:col + 256])
                    else:
                        nc.vector.tensor_copy(v_sb[:, tb * 256:(tb + 1) * 256],
                                              tile[:, col:col + 256])

            emit_kv_wave(0, [0, 1, 2, 3])
            emit_kv_wave(1, [4, 5, 6, 7])
            emit_kv_wave(None, [8, 9, 10, 11])
            emit_kv_wave(None, [12, 13, 14, 15])

            # wk/wv SBUF freed; wc streams in one slice per head below (DMA
            # is idle mid-kernel) so phase C can start the moment h7 finishes.
            kvctx.close()

            def emit_q_chunk(h, t4):
                # one quarter of head h's q projection: q^T[hd, t4*TQ:...]
                if t4 == 0:
                    qT_heads[h] = qpool.tile([128, T], FP16, name="qT", tag="qT")
                    wq_blocks[(h, "halves")] = [wq_blocks.pop(h * 2),
                                                wq_blocks.pop(h * 2 + 1)]
                    for hv in (h * 2 + 3, h * 2 + 4):
                        if 2 <= hv < HPC * 2 and hv not in wq_blocks:
                            wq_blocks[hv] = load_wq(hv)
                wq_halves = wq_blocks[(h, "halves")]
                ps_q = psS.tile([128, TQ], F32, name="qps", tag="sg")
                for cb in range(16):
                    nc.tensor.matmul(ps_q[:],
                                     lhsT=wq_halves[cb // 8][:, cb % 8, :],
                                     rhs=xs(cb, t4 * TQ, (t4 + 1) * TQ),
                                     start=(cb == 0), stop=(cb == 15))
                nc.vector.tensor_copy(
                    qT_heads[h][:, t4 * TQ:(t4 + 1) * TQ], ps_q[:])
                if t4 == NTQ - 1:
                    wq_blocks.pop((h, "halves"))

            for t4 in range(NTQ):
                emit_q_chunk(0, t4)

            for h in range(HPC - 1):
                # sync queue is backed up behind the x tiles, so these run
                # only after phase A's HBM-critical stretch drains
                nc.sync.dma_start(wc_sb[h][:], io["wc"][h * 128:(h + 1) * 128, :])
                for qt in range(NTQ):
                    emit_q_chunk(h + 1, qt)
                    emit_att_qt(h, qt)
                qT_heads.pop(h)
            nc.sync.dma_start(wc_sb[HPC - 1][:],
                              io["wc"][(HPC - 1) * 128:HPC * 128, :])

        # head-7 attention fused with phase C: c_proj matmuls fill the PE
        # while the Act engine runs h7's exps. psS (6 banks) swaps for
        # psS2 (4) + psC (2).
        psSctx.close()
        psS2 = psctx.enter_context(tc.tile_pool(name="psS2", bufs=2, space="PSUM"))
        psC = psctx.enter_context(tc.tile_pool(name="psC", bufs=2, space="PSUM"))
        sgpool[0] = psS2
        ostage = ctx.enter_context(tc.tile_pool(name="ostage", bufs=2))

        def emit_cproj_tq(tq, wide=False):
            # one full chain + its copy at a time: the copy of chain N runs
            # under chain N+1's matmuls, so psC bufs=2 never stalls the PE.
            # wide=True (tail, attention done) adds psY/psRS as extra banks.
            ost = ostage.tile([128, C], FP16, name="ost", tag="ost")
            for cp in range(4):
                if wide and cp == 2:
                    op = psY.tile([128, TQ], F32, name="vy", tag="yps")
                elif wide and cp == 3:
                    op = psRS.tile([128, TQ], F32, name="vr", tag="rsps")
                else:
                    op = psC.tile([128, 512], F32, name="ops", tag="ops")
                for h in range(HPC):
                    nc.tensor.matmul(
                        op[:],
                        lhsT=yT_all[h][:, tq * 128:(tq + 1) * 128],
                        rhs=wc_sb[h][:, cp * 512:(cp + 1) * 512],
                        start=(h == 0), stop=(h == HPC - 1))
                if cp % 2 == 0:
                    nc.scalar.copy(ost[:, cp * 512:(cp + 1) * 512], op[:])
                else:
                    nc.vector.tensor_copy(ost[:, cp * 512:(cp + 1) * 512], op[:])
                if cp == 1:
                    nc.scalar.dma_start(io["out"][tq * 128:(tq + 1) * 128, 0:1024],
                                        ost[:, 0:1024])
            nc.scalar.dma_start(io["out"][tq * 128:(tq + 1) * 128, 1024:2048],
                                ost[:, 1024:2048])

        for qt in range(NTQ):
            emit_att_qt(HPC - 1, qt)
            flush_pending()
            for tq in range(qt * 4, qt * 4 + 4):
                emit_cproj_tq(tq, wide=(qt == NTQ - 1))
        psctx.close()


def _build_nc():
    import concourse.tile as tile
    from concourse import bacc, mybir

    F32 = mybir.dt.float32
    FP16 = mybir.dt.float16
    nc = bacc.Bacc("TRN2", target_bir_lowering=False, debug=False,
                   num_devices=NCORES)
    io = {
        "xT": nc.dram_tensor("xT", [128, 16 * T], FP16, kind="ExternalInput").ap(),
        "wq": nc.dram_tensor("wq", [C, HPC * 128], FP16, kind="ExternalInput").ap(),
        "wk": nc.dram_tensor("wk", [128, 16 * 256], FP16, kind="ExternalInput").ap(),
        "wv": nc.dram_tensor("wv", [128, 16 * 256], FP16, kind="ExternalInput").ap(),
        "wc": nc.dram_tensor("wc", [HPC * 128, C], FP16, kind="ExternalInput").ap(),
        "maskA": nc.dram_tensor("maskA", [128, 896], FP16, kind="ExternalInput").ap(),
        "maskB": nc.dram_tensor("maskB", [128, 384], FP16, kind="ExternalInput").ap(),
        "ones": nc.dram_tensor("ones", [128, 128], FP16, kind="ExternalInput").ap(),
        "out": nc.dram_tensor("out", [T, C], FP16, kind="ExternalOutput").ap(),
    }
    with tile.TileContext(nc) as tc:
        _emit(tc, io)
    nc.compile()
    return nc


def _get_nc():
    global _NC
    if _NC is None:
        _NC = _build_nc()
    return _NC


def _prep_wq(WqS):
    """[2048, 1024] -> per-hv blocks [128, 8*128] so each load_wq DMA is
    one contiguous [128, 1024] slab (2KB SBUF lines, no descriptor storm)."""
    A = WqS.reshape(16, 128, 1024)
    blocks = []
    for hv in range(16):
        hd, half = divmod(hv, 2)
        blk = A[half * 8:(half + 1) * 8, :, hd * 128:(hd + 1) * 128]
        blocks.append(np.ascontiguousarray(blk.transpose(1, 0, 2)).reshape(128, 1024))
    return np.concatenate(blocks, axis=0)


def _prep_kv(W):
    """[2048, 256] -> [128, 16*256]: SBUF layout precomputed on host."""
    return np.ascontiguousarray(W.reshape(16, 128, 256).transpose(1, 0, 2)).reshape(128, 4096)


def make_in_maps(x, Wq, Wkv, Wc):
    BF = np.float16
    x = np.asarray(x, dtype=np.float32)
    Wq = np.asarray(Wq, dtype=np.float32)
    Wkv = np.asarray(Wkv, dtype=np.float32)
    Wc = np.asarray(Wc, dtype=np.float32)
    maskA, maskB = _make_masks()
    ones = np.ones((128, 128), dtype=BF)
    in_maps = []
    for core in range(NCORES):
        b, s = core // 2, core % 2
        in_maps.append({
            "xT": np.ascontiguousarray(
                x[b].T.reshape(16, 128, T).transpose(1, 0, 2)).reshape(128, 16 * T).astype(BF),
            "wq": _prep_wq(Wq[:, s * 1024:(s + 1) * 1024]).astype(BF),
            "wk": _prep_kv(Wkv[:, s * 256:(s + 1) * 256]).astype(BF),
            "wv": _prep_kv(Wkv[:, 512 + s * 256:512 + (s + 1) * 256]).astype(BF),
            "wc": Wc[s * 1024:(s + 1) * 1024, :].astype(BF),
            "maskA": maskA,
            "maskB": maskB,
            "ones": ones,
        })
    return in_maps


def combine_outputs(results, bc):
    bc = np.asarray(bc, dtype=np.float32)
    out = np.empty((B, T, C), dtype=np.float32)
    for b in range(B):
        out[b] = (results[2 * b]["out"].astype(np.float32)
                  + results[2 * b + 1]["out"].astype(np.float32))
    out += bc[None, None, :]
    return out


def kernel(x, Wq, Wkv, Wc, bc):
    from concourse.bass_utils import run_bass_kernel_spmd

    nc = _get_nc()
    in_maps = make_in_maps(x, Wq, Wkv, Wc)
    res = run_bass_kernel_spmd(nc, in_maps, list(range(NCORES)))
    return combine_outputs(res.results, bc)



# revision 71
# speedup vs baseline: 2.3422x; 1.0054x over previous
"""Causal GQA self-attention (B=4, T=2048, C=2048, H=16, HKV=4, D=128) on 8 trn2 cores.

Sharding: core = (batch b = core//2) x (kv-head pair s = core%2).
Each core computes, for its batch and its 2 kv heads (8 q heads), all-fp16
matmul inputs throughout:
  k/v projections in cb-major waves over 8 bank-exclusive PSUM chains so the
  projection work is absorbed into the x DMA window; q^T per head
  (SBUF-resident, 2-head rotation); causal attention in transposed layout
  (S^T blocks [tk=128, tq=512]) with the diagonal band narrowed to its valid
  widths (512/384/256/128); exp on Act with fp16 probs; post-exp
  multiplicative triangle masks on DVE; AV in fp16; row sums via fp16 DVE
  accumulation + ones[128,128]-matmuls (fused reduce+broadcast) and
  reciprocal_approx_fast; head-7 attention fused with the c_proj so its exp
  work hides under c_proj matmuls -> [T, C] fp16 partial output.
Host sums the two partials per batch and adds bc. All weights/x are
pre-transposed host-side so every DMA is contiguous (no descriptor storms).
Attention for head h is interleaved with the q-projection of head h+1 so the
Act-engine exp work hides under projection matmuls. GpSimd is kept idle
(its ucode library reloads stall all engines ~7us each).
"""

import math
from contextlib import ExitStack

import numpy as np

B, T, C = 4, 2048, 2048
HKV, D, G = 4, 128, 4
NCORES = 8
HPC = 8            # q heads per core
KVPC = 2           # kv heads per core
TQ = 512           # q-tile (free dim of S^T blocks)
NTQ = T // TQ      # 4
NKB = T // 128     # 16 k-blocks
SCALE = 1.0 / math.sqrt(D)

_NC = None


def _make_masks():
    """Multiplicative 0/1 masks for the narrowed diagonal band, applied to
    probs after exp. Each kb sub-block is computed at its valid width with a
    tq offset that aligns its leading 128x128 triangle: 1 iff col >= row."""
    i = np.arange(128)[:, None]

    def band(w):
        c = np.arange(w)[None, :]
        return (c >= i).astype(np.float16)

    maskA = np.concatenate([band(512), band(384)], axis=1)   # p0 | p1
    maskB = np.concatenate([band(256), band(128)], axis=1)   # p2 | p3
    return maskA, maskB


def _emit(tc, io):
    from concourse import mybir

    nc = tc.nc
    F32 = mybir.dt.float32
    FP16 = mybir.dt.float16
    EXP = mybir.ActivationFunctionType.Exp
    ADD = mybir.AluOpType.add
    MULT = mybir.AluOpType.mult

    ctx = ExitStack()
    with ctx:
        persist = ctx.enter_context(tc.tile_pool(name="persist", bufs=1))
        kT_sb = persist.tile([128, KVPC * T], FP16, name="kT", tag="kT")   # [d, kv*T + t]
        v_sb = persist.tile([128, NKB * 256], FP16, name="v", tag="v")     # [t%128, tb*256 + kv*128 + d]
        ones_sb = persist.tile([128, 128], FP16, name="ones", tag="ones")
        maskA = persist.tile([128, 896], FP16, name="maskA", tag="maskA")
        maskB = persist.tile([128, 384], FP16, name="maskB", tag="maskB")
        nc.scalar.dma_start(maskA[:], io["maskA"])
        nc.scalar.dma_start(maskB[:], io["maskB"])
        nc.scalar.dma_start(ones_sb[:], io["ones"])

        # long-lived attention-side pools
        yres = ctx.enter_context(tc.tile_pool(name="yres", bufs=8))
        yT_all = [yres.tile([128, T], FP16, name="yT", tag="yT") for _ in range(HPC)]
        qpool = ctx.enter_context(tc.tile_pool(name="qpool", bufs=2))
        pTpool = ctx.enter_context(tc.tile_pool(name="pTp", bufs=4))
        rsaccp = ctx.enter_context(tc.tile_pool(name="rsaccp", bufs=2))
        binvp = ctx.enter_context(tc.tile_pool(name="binvp", bufs=2))

        # PSUM pools (8 banks total): psY 1 + psRS 1 + psS 3x[128,1024] (6).
        # psS opens last so it can be swapped for psS2(4) + psC(2) at head 7.
        psctx = ExitStack()
        psY = psctx.enter_context(tc.tile_pool(name="psY", bufs=1, space="PSUM"))
        psRS = psctx.enter_context(tc.tile_pool(name="psRS", bufs=1, space="PSUM"))
        psSctx = ExitStack()
        psS = psSctx.enter_context(tc.tile_pool(name="psS", bufs=3, space="PSUM"))
        sgpool = [psS]

        qT_heads = {}
        wq_blocks = {}

        # ---------------- attention emission helpers ----------------
        pending = []   # deferred tail work, flushed inside the next block
        filler_gen = [None]   # generator yielding PE filler chunks (fused C)

        def flush_pending():
            while pending:
                pending.pop(0)()

        def run_filler(n=1):
            gen = filler_gen[0]
            if gen is None:
                return
            for _ in range(n):
                try:
                    next(gen)
                except StopIteration:
                    filler_gen[0] = None
                    return

        def emit_rs_finish(rs_acc, h, qt):
            # ones[128,128]-matmul: every PSUM partition gets the full rowsum
            # (fused reduce + partition-broadcast), then reciprocal + normalize.
            def work():
                rs_ps = psRS.tile([128, TQ], F32, name="rsps", tag="rsps")
                if qt == 0:
                    # half1 cols 0:128 were never written (narrowed diagonal)
                    nc.tensor.matmul(rs_ps[:], lhsT=ones_sb[:],
                                     rhs=rs_acc[:, 0:TQ],
                                     start=True, stop=False)
                    nc.tensor.matmul(rs_ps[:, 128:TQ], lhsT=ones_sb[:],
                                     rhs=rs_acc[:, TQ + 128:2 * TQ],
                                     start=False, stop=True)
                else:
                    for j in range(2):
                        nc.tensor.matmul(rs_ps[:],
                                         lhsT=ones_sb[:],
                                         rhs=rs_acc[:, j * TQ:(j + 1) * TQ],
                                         start=(j == 0), stop=(j == 1))
                binv = binvp.tile([128, TQ], F32, name="binv", tag="binv")
                nc.vector.reciprocal_approx_fast(binv[:], rs_ps[:])
                nc.vector.tensor_tensor(
                    out=yT_all[h][:, qt * TQ:(qt + 1) * TQ],
                    in0=yT_all[h][:, qt * TQ:(qt + 1) * TQ],
                    in1=binv[:], op=MULT)
            pending.append(work)

        def emit_att_qt(h, qt):
            kv = h // G
            qT_h = qT_heads[h]
            nkb = (qt + 1) * (TQ // 128)
            ng = nkb // 2
            y_ps = psY.tile([128, TQ], F32, name="yps", tag="yps")
            rs_acc = rsaccp.tile([128, 1024], FP16, name="rsacc", tag="rsacc")

            sgs = {}
            pTs = {}

            def parts_of(g):
                # (kb, col0, width, tq_off, rs_col0) per kb sub-block; the
                # last two groups are the causal diagonal band, narrowed to
                # their valid widths (fp16 matmul is full-rate at any width).
                if g == ng - 2:
                    return [(4 * qt + 0, 0, 512, 0, 0),
                            (4 * qt + 1, 512, 384, 128, TQ + 128)], maskA
                if g == ng - 1:
                    return [(4 * qt + 2, 0, 256, 256, 256),
                            (4 * qt + 3, 256, 128, 384, TQ + 384)], maskB
                return [(2 * g, 0, 512, 0, 0),
                        (2 * g + 1, 512, 512, 0, TQ)], None

            def emit_S(g):
                parts, _ = parts_of(g)
                sg = sgpool[0].tile([128, 1024], F32, name="sg", tag="sg")
                for kb, col0, w, tq_off, _rs in parts:
                    nc.tensor.matmul(
                        sg[:, col0:col0 + w],
                        lhsT=kT_sb[:, kv * T + kb * 128:kv * T + (kb + 1) * 128],
                        rhs=qT_h[:, qt * TQ + tq_off:(qt + 1) * TQ],
                        start=True, stop=True)
                sgs[g] = sg

            def emit_E(g):
                parts, m = parts_of(g)
                W = parts[-1][1] + parts[-1][2]
                pT = pTpool.tile([128, 1024], FP16, name="pT", tag="pT")
                nc.scalar.activation(pT[:, 0:W], sgs.pop(g)[:, 0:W], EXP,
                                     scale=SCALE)
                if m is not None:        # diagonal band: zero invalid probs
                    nc.vector.tensor_tensor(out=pT[:, 0:W], in0=pT[:, 0:W],
                                            in1=m[:], op=MULT)
                pTs[g] = pT

            def emit_AV(g):
                parts, _ = parts_of(g)
                pT = pTs.pop(g)
                for kb, col0, w, tq_off, _rs in parts:
                    nc.tensor.matmul(
                        y_ps[:, tq_off:TQ],
                        lhsT=v_sb[:, kb * 256 + kv * 128:kb * 256 + (kv + 1) * 128],
                        rhs=pT[:, col0:col0 + w],
                        start=(kb == 0), stop=(kb == nkb - 1))
                for kb, col0, w, tq_off, rs0 in parts:
                    if g == 0 and kb < 2:
                        nc.vector.tensor_copy(rs_acc[:, rs0:rs0 + w],
                                              pT[:, col0:col0 + w])
                    else:
                        nc.vector.tensor_tensor(
                            out=rs_acc[:, rs0:rs0 + w],
                            in0=rs_acc[:, rs0:rs0 + w],
                            in1=pT[:, col0:col0 + w], op=ADD)

            emit_S(0)
            emit_E(0)
            flush_pending()
            run_filler()
            if ng > 1:
                emit_S(1)
                emit_E(1)
                run_filler()
            for g in range(2, ng):
                emit_S(g)
                emit_AV(g - 2)
                emit_E(g)
                run_filler()
            if ng > 1:
                emit_AV(ng - 2)
            emit_AV(ng - 1)
            nc.vector.tensor_copy(yT_all[h][:, qt * TQ:(qt + 1) * TQ], y_ps[:])
            emit_rs_finish(rs_acc, h, qt)

        # ---------------- Phase A + interleaved attention ----------------
        wcres = ctx.enter_context(tc.tile_pool(name="wcres", bufs=1))
        wc_sb = [wcres.tile([128, C], FP16, name="wc", tag=f"wc{h}")
                 for h in range(HPC)]

        with (
            tc.tile_pool(name="xpool", bufs=1) as xpool,
            tc.tile_pool(name="wstream", bufs=3) as wstream,
        ):
            kvctx = ExitStack()
            wkres = kvctx.enter_context(tc.tile_pool(name="wkres", bufs=1))
            wvres = kvctx.enter_context(tc.tile_pool(name="wvres", bufs=1))

            def load_wq(hv):
                t = wstream.tile([128, 8, 128], FP16, name="wqb", tag="wqb", bufs=3)
                nc.sync.dma_start(
                    t[:].rearrange("r a c -> r (a c)"),
                    io["wq"][hv * 128:(hv + 1) * 128, :])
                return t

            # (host pre-transposes wk/wv/wq so these DMAs are contiguous)
            wk_sb = wkres.tile([128, 16 * 256], FP16, name="wk", tag="wk")



            # x: 16 per-tile DMAs in consumption order on one queue; per-tile
            # completion keeps the kproj chain pipelined with the load.
            wv_sb = wvres.tile([128, 16 * 256], FP16, name="wv", tag="wv")
            nc.gpsimd.dma_start(wv_sb[:], io["wv"])
            x_sb = []
            for cb in range(16):
                xt = xpool.tile([128, T], FP16, name="x", tag=f"x{cb}")
                x_sb.append(xt)
            nc.sync.dma_start(x_sb[0][:], io["xT"][:, 0:T])
            nc.sync.dma_start(wk_sb[:], io["wk"])
            for cb in range(1, 16):
                nc.sync.dma_start(x_sb[cb][:], io["xT"][:, cb * T:(cb + 1) * T])

            def xs(cb, lo, hi):
                return x_sb[cb][:, lo:hi]

            for i in range(3):
                wq_blocks[i] = load_wq(i)

            # k+v projections in waves, cb-major: 8 concurrent bank-exclusive
            # PSUM chains consume each x tile as it lands, absorbing most of
            # the projection work into the x DMA window.
            def emit_kv_wave(kvi, tbs):
                kp = ([psS.tile([128, 1024], F32, name="kp", tag="sg")
                       for _ in range(2)] if kvi is not None else [])
                vp = psS.tile([128, 1024], F32, name="vp", tag="sg")
                vy = psY.tile([128, TQ], F32, name="vy", tag="yps")
                vr = psRS.tile([128, TQ], F32, name="vr", tag="rsps")
                vslots = [(vp, 0), (vp, 512), (vy, 0), (vr, 0)]
                for cb in range(16):
                    for i, tb in enumerate(tbs):
                        tile, col = vslots[i]
                        nc.tensor.matmul(
                            tile[:, col:col + 256],
                            lhsT=xs(cb, tb * 128, (tb + 1) * 128),
                            rhs=wv_sb[:, cb * 256:(cb + 1) * 256],
                            start=(cb == 0), stop=(cb == 15))
                    if kvi is not None:
                        for t4 in range(NTQ):
                            tile, col = kp[t4 // 2], (t4 % 2) * 512
                            nc.tensor.matmul(
                                tile[:, col:col + 512],
                                lhsT=wk_sb[:, cb * 256 + kvi * 128:cb * 256 + (kvi + 1) * 128],
                                rhs=xs(cb, t4 * TQ, (t4 + 1) * TQ),
                                start=(cb == 0), stop=(cb == 15))
                if kvi is not None:
                    for t4 in range(NTQ):
                        tile, col = kp[t4 // 2], (t4 % 2) * 512
                        nc.vector.tensor_copy(
                            kT_sb[:, kvi * T + t4 * TQ:kvi * T + (t4 + 1) * TQ],
                            tile[:, col:col + 512])
                for i, tb in enumerate(tbs):
                    tile, col = vslots[i]
                    if i % 2:
                        nc.scalar.copy(v_sb[:, tb * 256:(tb + 1) * 256],
                                       tile[:, col:col + 256])
                    else:
                        nc.vector.tensor_copy(v_sb[:, tb * 256:(tb + 1) * 256],
                                              tile[:, col:col + 256])

            emit_kv_wave(0, [0, 1, 2, 3])
            emit_kv_wave(1, [4, 5, 6, 7])
            emit_kv_wave(None, [8, 9, 10, 11])
            emit_kv_wave(None, [12, 13, 14, 15])

            # wk/wv SBUF freed; wc streams in one slice per head below (DMA
            # is idle mid-kernel) so phase C can start the moment h7 finishes.
            kvctx.close()

            def emit_q_chunk(h, t4):
                # one quarter of head h's q projection: q^T[hd, t4*TQ:...]
                if t4 == 0:
                    qT_heads[h] = qpool.tile([128, T], FP16, name="qT", tag="qT")
                    wq_blocks[(h, "halves")] = [wq_blocks.pop(h * 2),
                                                wq_blocks.pop(h * 2 + 1)]
                    for hv in (h * 2 + 3, h * 2 + 4):
                        if 2 <= hv < HPC * 2 and hv not in wq_blocks:
                            wq_blocks[hv] = load_wq(hv)
                wq_halves = wq_blocks[(h, "halves")]
                ps_q = psS.tile([128, TQ], F32, name="qps", tag="sg")
                for cb in range(16):
                    nc.tensor.matmul(ps_q[:],
                                     lhsT=wq_halves[cb // 8][:, cb % 8, :],
                                     rhs=xs(cb, t4 * TQ, (t4 + 1) * TQ),
                                     start=(cb == 0), stop=(cb == 15))
                nc.vector.tensor_copy(
                    qT_heads[h][:, t4 * TQ:(t4 + 1) * TQ], ps_q[:])
                if t4 == NTQ - 1:
                    wq_blocks.pop((h, "halves"))

            for t4 in range(NTQ):
                emit_q_chunk(0, t4)

            for h in range(HPC - 1):
                # sync queue is backed up behind the x tiles, so these run
                # only after phase A's HBM-critical stretch drains
                nc.sync.dma_start(wc_sb[h][:], io["wc"][h * 128:(h + 1) * 128, :])
                for qt in range(NTQ):
                    emit_q_chunk(h + 1, qt)
                    emit_att_qt(h, qt)
                qT_heads.pop(h)
            nc.sync.dma_start(wc_sb[HPC - 1][:],
                              io["wc"][(HPC - 1) * 128:HPC * 128, :])

        # head-7 attention fused with phase C: c_proj matmuls fill the PE
        # while the Act engine runs h7's exps. psS (6 banks) swaps for
        # psS2 (4) + psC (2).
        psSctx.close()
        psS2 = psctx.enter_context(tc.tile_pool(name="psS2", bufs=2, space="PSUM"))
        psC = psctx.enter_context(tc.tile_pool(name="psC", bufs=2, space="PSUM"))
        sgpool[0] = psS2
        ostage = ctx.enter_context(tc.tile_pool(name="ostage", bufs=2))

        def emit_cproj_tq(tq, wide=False):
            # one full chain + its copy at a time: the copy of chain N runs
            # under chain N+1's matmuls, so psC bufs=2 never stalls the PE.
            # wide=True (tail, attention done) adds psY/psRS as extra banks.
            ost = ostage.tile([128, C], FP16, name="ost", tag="ost")
            for cp in range(4):
                if wide and cp == 2:
                    op = psY.tile([128, TQ], F32, name="vy", tag="yps")
                elif wide and cp == 3:
                    op = psRS.tile([128, TQ], F32, name="vr", tag="rsps")
                else:
                    op = psC.tile([128, 512], F32, name="ops", tag="ops")
                for h in range(HPC):
                    nc.tensor.matmul(
                        op[:],
                        lhsT=yT_all[h][:, tq * 128:(tq + 1) * 128],
                        rhs=wc_sb[h][:, cp * 512:(cp + 1) * 512],
                        start=(h == 0), stop=(h == HPC - 1))
                if cp % 2 == 0:
                    nc.scalar.copy(ost[:, cp * 512:(cp + 1) * 512], op[:])
                else:
                    nc.vector.tensor_copy(ost[:, cp * 512:(cp + 1) * 512], op[:])
                if cp == 1:
                    nc.scalar.dma_start(io["out"][tq * 128:(tq + 1) * 128, 0:1024],
                                        ost[:, 0:1024])
            nc.scalar.dma_start(io["out"][tq * 128:(tq + 1) * 128, 1024:2048],
                                ost[:, 1024:2048])

        def cproj_gen(tqs):
            # per-chain generator: each next() emits one 8-matmul c_proj
            # chain + its copy, used as PE filler inside att(7, qt)
            for tq in tqs:
                ost = ostage.tile([128, C], FP16, name="ost", tag="ost")
                for cp in range(4):
                    op = psC.tile([128, 512], F32, name="ops", tag="ops")
                    for h in range(HPC):
                        nc.tensor.matmul(
                            op[:],
                            lhsT=yT_all[h][:, tq * 128:(tq + 1) * 128],
                            rhs=wc_sb[h][:, cp * 512:(cp + 1) * 512],
                            start=(h == 0), stop=(h == HPC - 1))
                    if cp % 2 == 0:
                        nc.scalar.copy(ost[:, cp * 512:(cp + 1) * 512], op[:])
                    else:
                        nc.vector.tensor_copy(ost[:, cp * 512:(cp + 1) * 512],
                                              op[:])
                    if cp == 1:
                        nc.scalar.dma_start(
                            io["out"][tq * 128:(tq + 1) * 128, 0:1024],
                            ost[:, 0:1024])
                    yield
                nc.scalar.dma_start(io["out"][tq * 128:(tq + 1) * 128, 1024:2048],
                                    ost[:, 1024:2048])

        for qt in range(NTQ):
            if qt > 0:
                filler_gen[0] = cproj_gen(range((qt - 1) * 4, qt * 4))
            emit_att_qt(HPC - 1, qt)
            flush_pending()
            while filler_gen[0] is not None:
                run_filler()
        for tq in range((NTQ - 1) * 4, NTQ * 4):
            emit_cproj_tq(tq, wide=True)
        psctx.close()


def _build_nc():
    import concourse.tile as tile
    from concourse import bacc, mybir

    F32 = mybir.dt.float32
    FP16 = mybir.dt.float16
    nc = bacc.Bacc("TRN2", target_bir_lowering=False, debug=False,
                   num_devices=NCORES)
    io = {
        "xT": nc.dram_tensor("xT", [128, 16 * T], FP16, kind="ExternalInput").ap(),
        "wq": nc.dram_tensor("wq", [C, HPC * 128], FP16, kind="ExternalInput").ap(),
        "wk": nc.dram_tensor("wk", [128, 16 * 256], FP16, kind="ExternalInput").ap(),
        "wv": nc.dram_tensor("wv", [128, 16 * 256], FP16, kind="ExternalInput").ap(),
        "wc": nc.dram_tensor("wc", [HPC * 128, C], FP16, kind="ExternalInput").ap(),
        "maskA": nc.dram_tensor("maskA", [128, 896], FP16, kind="ExternalInput").ap(),
        "maskB": nc.dram_tensor("maskB", [128, 384], FP16, kind="ExternalInput").ap(),
        "ones": nc.dram_tensor("ones", [128, 128], FP16, kind="ExternalInput").ap(),
        "out": nc.dram_tensor("out", [T, C], FP16, kind="ExternalOutput").ap(),
    }
    with tile.TileContext(nc) as tc:
        _emit(tc, io)
    nc.compile()
    return nc


def _get_nc():
    global _NC
    if _NC is None:
        _NC = _build_nc()
    return _NC


def _prep_wq(WqS):
    """[2048, 1024] -> per-hv blocks [128, 8*128] so each load_wq DMA is
    one contiguous [128, 1024] slab (2KB SBUF lines, no descriptor storm)."""
    A = WqS.reshape(16, 128, 1024)
    blocks = []
    for hv in range(16):
        hd, half = divmod(hv, 2)
        blk = A[half * 8:(half + 1) * 8, :, hd * 128:(hd + 1) * 128]
        blocks.append(np.ascontiguousarray(blk.transpose(1, 0, 2)).reshape(128, 1024))
    return np.concatenate(blocks, axis=0)


def _prep_kv(W):
    """[2048, 256] -> [128, 16*256]: SBUF layout precomputed on host."""
    return np.ascontiguousarray(W.reshape(16, 128, 256).transpose(1, 0, 2)).reshape(128, 4096)


def make_in_maps(x, Wq, Wkv, Wc):
    BF = np.float16
    x = np.asarray(x, dtype=np.float32)
    Wq = np.asarray(Wq, dtype=np.float32)
    Wkv = np.asarray(Wkv, dtype=np.float32)
    Wc = np.asarray(Wc, dtype=np.float32)
    maskA, maskB = _make_masks()
    ones = np.ones((128, 128), dtype=BF)
    in_maps = []
    for core in range(NCORES):
        b, s = core // 2, core % 2
        in_maps.append({
            "xT": np.ascontiguousarray(
                x[b].T.reshape(16, 128, T).transpose(1, 0, 2)).reshape(128, 16 * T).astype(BF),
            "wq": _prep_wq(Wq[:, s * 1024:(s + 1) * 1024]).astype(BF),
            "wk": _prep_kv(Wkv[:, s * 256:(s + 1) * 256]).astype(BF),
            "wv": _prep_kv(Wkv[:, 512 + s * 256:512 + (s + 1) * 256]).astype(BF),
            "wc": Wc[s * 1024:(s + 1) * 1024, :].astype(BF),
            "maskA": maskA,
            "maskB": maskB,
            "ones": ones,
        })
    return in_maps


def combine_outputs(results, bc):
    bc = np.asarray(bc, dtype=np.float32)
    out = np.empty((B, T, C), dtype=np.float32)
    for b in range(B):
        out[b] = (results[2 * b]["out"].astype(np.float32)
                  + results[2 * b + 1]["out"].astype(np.float32))
    out += bc[None, None, :]
    return out


def kernel(x, Wq, Wkv, Wc, bc):
    from concourse.bass_utils import run_bass_kernel_spmd

    nc = _get_nc()
    in_maps = make_in_maps(x, Wq, Wkv, Wc)
    res = run_bass_kernel_spmd(nc, in_maps, list(range(NCORES)))
    return combine_outputs(res.results, bc)

